# revision 16
# baseline (speedup 1.0000x reference)
"""Trainium2 Bass kernel for the class-balanced supervised-contrastive loss.

Math (reference semantics, shift-invariant form with constant shift 10):
  l_ij = (f_i . g_j) / T,  T = 0.1, g = [features; centers; features_ood]
  E_ij = exp(l_ij - 10)
  S_i  = sum_{j != i} E_ij / (w_j - eq_ij)        (w_j = class count, eq = label match)
  P_i  = sum_{j != i} eq_ij (l_ij - 10)
  loss = -mean_i( P_i / K_i - log S_i ),  K_i = batch count of class t_i

Device per row (tolerance is 2e-2 rel, so a single fp8 pass suffices —
validated 5e-5 end-to-end on the reference data):
  psum_ij = 256*(r_ij + bias1_j),  bias1_j = -(ln w_j + 10)/10
    r from one fp8(e4m3, inputs pre-scaled x16) DoubleRow matmul pair
    (K=256 per instruction, 0.5 PE cycles/row), bias via a K=2 bf16
    hi/lo ones-matmul into the same PSUM accumulation group.
  ACT: E1 = exp((10/256)*psum), accum_out -> A_i = sum_j E1  (all 18 chunks)
  DVE (window chunks only, where all label matches live after the host
  permutation): S2 = sum eq*E1, S3 = sum eq*psum, diag = psum_ii.
Everything else is O(B) host math.

Device layout per core (rows sharded, 512 rows/core): columns are permuted
to [own 512 rows | matched cols | rest | ood | pad] so every eq-match (and
the diagonal, at column 128m+p of chunk 0 for row-tile m partition p) is
confined to the first `eqw` (=2) 512-column chunks. Chunks are processed in
groups of 4 (one 4-bank PSUM tile, double-buffered) so each exp covers
[128, 2048] in one ACT instruction.
"""

import ml_dtypes
import numpy as np

import concourse.bass as bass
import concourse.mybir as mybir
import concourse.tile as tile
from concourse.bass_utils import run_bass_kernel_spmd

NCORES = 8
C, TEMP = 1000, 0.1
B, BO, D = 4096, 4096, 512
N = B + C + BO              # 9192
NPAD = 9216                 # 18 * 512
PAD = NPAD - N
NCH = NPAD // 512           # 18 column chunks
RPC = B // NCORES           # 512 rows per core
MT = RPC // 128             # 4 row tiles per core
SCALE = 16.0                # fp8 operand pre-scale; psum carries 256*(r+bias)
# chunk groups per ACT instruction; the window group (chunks 0..1, which
# feeds the extra DVE reductions) goes LAST so its psum->DVE->ACT chain
# hides under the other quads' ACT work at each m boundary
GRPS = [(2, 6), (6, 10), (10, 14), (14, 18), (0, 2)]

F32 = mybir.dt.float32
BF16 = mybir.dt.bfloat16
F8 = mybir.dt.float8e4
ALU = mybir.AluOpType
AF = mybir.ActivationFunctionType
BFNP = ml_dtypes.bfloat16
F8NP = ml_dtypes.float8_e4m3
DR = mybir.MatmulPerfMode.DoubleRow

# This walrus build accepts only one sync-wait command per engine instruction.
# Move surplus waits onto standalone EventSemaphore instructions just before
# the affected instruction (same engine, so blocking semantics are identical).
_SPLIT_SKIP = ("InstEventSemaphore",)


def _split_multi_waits(nc):
    n = 0
    for f in nc.m.functions:
        for bb in f.blocks:
            new = []
            for ins in bb.instructions:
                si = ins.sync_info
                if (
                    si is not None
                    and si.on_wait
                    and len(si.on_wait) > 1
                    and type(ins).__name__ not in _SPLIT_SKIP
                ):
                    waits = list(si.on_wait)
                    for w in waits[:-1]:
                        es = mybir.InstEventSemaphore(
                            name=f"wsplit_{n}",
                            engine=ins.engine,
                            sync_info=mybir.SyncInfo(on_wait=[w], on_update=[]),
                        )
                        n += 1
                        new.append(es)
                    ins.sync_info = mybir.SyncInfo(
                        on_wait=[waits[-1]], on_update=list(si.on_update)
                    )
                new.append(ins)
            bb.instructions = new
    return n


def _build_nc(eqw=2, wneed=1024):
    assert eqw <= 2, "label window must fit the first (0,2) chunk group"
    wcols = wneed  # actual matched extent (<= eqw*512), 128-aligned
    nc = bass.Bass()
    # host pre-tiles to the SBUF layout: chunk ch at [128, 4, 512] block ch,
    # element (p, ks, j) = scaled g[col j of chunk][dim p + 128*ks]
    gT8 = nc.declare_dram_parameter("gT8", [128, NCH * 2048], F8, isOutput=False)
    fT8 = nc.declare_dram_parameter("fT8", [128, 2048], F8, isOutput=False)
    # [2, .]: row 0 = (ones128, bias_hi row), row 1 = lo parts
    cst = nc.declare_dram_parameter("cst", [2, 128 + NPAD], BF16, isOutput=False)
    ta = nc.declare_dram_parameter("ta", [128, wcols], F32, isOutput=False)
    tvec = nc.declare_dram_parameter("tvec", [128, MT], F32, isOutput=False)
    ident = nc.declare_dram_parameter("ident", [128, 128], F32, isOutput=False)
    out = nc.declare_dram_parameter("out", [128, 4 * MT], F32, isOutput=True)

    with tile.TileContext(nc) as tc:
        with (
            tc.tile_pool(name="const", bufs=1) as const,
            tc.tile_pool(name="stats", bufs=1) as stats,
            tc.tile_pool(name="scr", bufs=2) as scr,
            tc.tile_pool(name="psum", bufs=2, space="PSUM") as psp,
        ):
            # DMAs split across the SP and Pool (SWDGE) queues: the tile-sim
            # charges transfer time (free-dim bytes) serially per issuing
            # engine, so one queue would cost ~24us of latency.
            ft8 = const.tile([128, 4, 512], F8)
            nc.sync.dma_start(out=ft8[:], in_=fT8[:])
            cst_sb = const.tile([2, 128 + NPAD], BF16)
            g8 = const.tile([128, NCH * 4, 512], F8)

            def g8dma(eng, c0, c1):
                eng.dma_start(
                    out=g8[:, 4 * c0 : 4 * c1, :], in_=gT8[:, 2048 * c0 : 2048 * c1]
                )

            def cstdma(eng, a, b):
                nc_off = 128  # ones row precedes the biases
                eng.dma_start(
                    out=cst_sb[:, nc_off + 512 * a : nc_off + 512 * b],
                    in_=cst[:, nc_off + 512 * a : nc_off + 512 * b],
                )

            # DMAs split across the SP and Pool (SWDGE) queues — the tile-sim
            # charges transfer time (free-dim bytes) serially per issuing
            # engine — and sequenced so each piece lands just before the
            # group order [2-5, 6-9, 10-13, 14-17, 0-1] consumes it.
            nc.gpsimd.dma_start(out=cst_sb[:, 0:128], in_=cst[:, 0:128])  # ones
            cstdma(nc.gpsimd, 2, 6)
            g8dma(nc.sync, 2, 4)
            g8dma(nc.sync, 4, 6)
            g8dma(nc.gpsimd, 6, 8)
            g8dma(nc.gpsimd, 8, 10)
            g8dma(nc.sync, 10, 12)
            cstdma(nc.gpsimd, 6, 10)
            g8dma(nc.sync, 12, 14)
            cstdma(nc.gpsimd, 10, 14)
            tvec_sb = const.tile([128, MT], F32)
            nc.sync.dma_start(out=tvec_sb[:], in_=tvec[:])
            ident_sb = const.tile([128, 128], F32)
            nc.sync.dma_start(out=ident_sb[:], in_=ident[:])
            g8dma(nc.gpsimd, 14, 16)
            g8dma(nc.gpsimd, 16, 18)
            g8dma(nc.sync, 0, 2)
            cstdma(nc.sync, 14, 18)
            cstdma(nc.sync, 0, 2)
            ta_sb = const.tile([128, wcols], F32)
            nc.gpsimd.dma_start(out=ta_sb[:], in_=ta[:])
            ones_sb = cst_sb[:, 0:128]
            brow_sb = cst_sb[:, 128 : 128 + NPAD]

            outsb = stats.tile([128, 4 * MT], F32)
            a_slot = [stats.tile([128, len(GRPS)], F32, name=f"a{m}") for m in range(MT)]

            # warm the ACT Exp table during the DMA head so the first real
            # activation doesn't pay the 1.3us table load
            warm = stats.tile([2, 1], F32)
            nc.scalar.activation(warm[:], ones_sb[:, 0:1], AF.Exp, scale=1.0)

            for m in range(MT):
                for gi, (gs, ge) in enumerate(GRPS):
                    gw = (ge - gs) * 512
                    ps = psp.tile([128, 2048], F32)
                    for ch in range(gs, ge):
                        co = (ch - gs) * 512
                        pslice = ps[:, co : co + 512]
                        # bias matmul last so the fp8 work can start before
                        # the bias rows finish streaming in
                        for q in range(2):
                            nc.tensor.matmul(
                                pslice,
                                ft8[:, 2 * q : 2 * q + 2, 128 * m : 128 * (m + 1)],
                                g8[:, 4 * ch + 2 * q : 4 * ch + 2 * q + 2, :],
                                start=(q == 0),
                                stop=False,
                                perf_mode=DR,
                            )
                        nc.tensor.matmul(
                            pslice,
                            ones_sb,
                            brow_sb[:, 512 * ch : 512 * (ch + 1)],
                            start=False,
                            stop=True,
                        )
                    if gs == 0:
                        # psum readers (S3, diag) are emitted BEFORE the exp:
                        # tile accesses chain in emission order, so putting
                        # them first lets them start as soon as the PE's
                        # window psum is ready (the exp then chains after).
                        sc3 = scr.tile([128, wcols], F32, tag="scr3")
                        nc.vector.scalar_tensor_tensor(
                            out=sc3[:],
                            in0=ta_sb[:],
                            scalar=tvec_sb[:, m : m + 1],
                            in1=ps[:, :wcols],
                            op0=ALU.is_equal,
                            op1=ALU.mult,
                            accum_out=outsb[:, 4 * m + 2 : 4 * m + 3],
                        )
                        # local row p's own column is chunk-0 column 128m+p,
                        # so the psum diagonal of this [128,128] sub-block is
                        # the self dot-product (plus bias) bit-exactly.
                        sd = scr.tile([128, 128], F32, tag="scrd")
                        nc.vector.scalar_tensor_tensor(
                            out=sd[:],
                            in0=ident_sb[:],
                            scalar=1.0,
                            in1=ps[:, 128 * m : 128 * (m + 1)],
                            op0=ALU.mult,
                            op1=ALU.mult,
                            accum_out=outsb[:, 4 * m + 3 : 4 * m + 4],
                        )
                    # exp in place over the psum tile: saves the SBUF write
                    # (smaller ACT init) and the e1 buffers entirely; the
                    # window's S2 reads the exp'd psum afterwards
                    nc.scalar.activation(
                        ps[:, :gw],
                        ps[:, :gw],
                        AF.Exp,
                        scale=10.0 / 256.0,
                        accum_out=a_slot[m][:, gi : gi + 1],
                    )
                    if gs == 0:
                        sc = scr.tile([128, wcols], F32, tag="scr2")
                        nc.vector.scalar_tensor_tensor(
                            out=sc[:],
                            in0=ta_sb[:],
                            scalar=tvec_sb[:, m : m + 1],
                            in1=ps[:, :wcols],
                            op0=ALU.is_equal,
                            op1=ALU.mult,
                            accum_out=outsb[:, 4 * m + 1 : 4 * m + 2],
                        )
                nc.vector.tensor_reduce(
                    outsb[:, 4 * m : 4 * m + 1],
                    a_slot[m][:],
                    mybir.AxisListType.X,
                    ALU.add,
                )
            nc.sync.dma_start(out=out[:], in_=outsb[:])
    _split_multi_waits(nc)
    return nc


_nc_by_cfg = {}


def _get_nc(eqw, wneed):
    key = (eqw, wneed)
    if key not in _nc_by_cfg:
        _nc_by_cfg[key] = _build_nc(eqw, wneed)
    return _nc_by_cfg[key]


def _prepare(centers1, features, targets, features_ood, pseudo_target_ood):
    """Host-side O(N log N) prep.

    Rows are globally sorted by class and sharded contiguously, so each
    core's 512 rows cover ~C/8 classes whose other members mostly live in
    the same core. Per core the g columns are permuted to
    [own 512 rows | all other same-class batch cols + own-class centers |
     rest bc cols | ood | pad], which confines every eq-match (and the
    diagonal, at column 128m+p for row-tile m partition p) to the first
    eqw chunks. Only those chunks need the masked S2/S3 reductions.
    """
    centers1 = np.asarray(centers1, np.float32)
    features = np.asarray(features, np.float32)
    features_ood = np.asarray(features_ood, np.float32)
    targets = np.asarray(targets).astype(np.int64)
    pseudo = np.asarray(pseudo_target_ood).astype(np.int64)

    tac = np.concatenate([targets, np.arange(C), pseudo])
    w_full = np.bincount(tac, minlength=C).astype(np.float64)

    # class-id label per g row (incl. centers/ood), and bias per g row
    lab = np.concatenate([targets, np.arange(C), np.full(BO, C, np.int64),
                          np.full(PAD, -1, np.int64)])
    bias1 = np.full(NPAD, -20.0 * 256.0, np.float64)
    bias1[:N] = -(np.log(w_full[tac]) + 10.0) / 10.0 * 256.0
    b_h = bias1.astype(BFNP)
    b_l = (bias1 - b_h.astype(np.float64)).astype(BFNP)

    g = np.concatenate(
        [features, centers1, features_ood, np.zeros((PAD, D), np.float32)], axis=0
    )
    g8 = (g * SCALE).astype(F8NP)

    row_perm = np.argsort(targets, kind="stable")
    t_sorted = targets[row_perm]

    ident = np.eye(128, dtype=np.float32)
    ones2 = np.ones((2, 128), BFNP)

    # per-core column permutations
    perms = []
    eqw_need = 1
    mm_need = RPC + 1
    all_batch = np.arange(B)
    for c in range(NCORES):
        own = row_perm[RPC * c : RPC * (c + 1)]            # sorted by class
        tset = np.zeros(C + 1, bool)
        tset[t_sorted[RPC * c : RPC * (c + 1)]] = True
        in_own = np.zeros(B, bool)
        in_own[own] = True
        match_b = all_batch[tset[targets] & ~in_own]       # other cores' rows, own classes
        match_c = B + np.flatnonzero(tset[:C])             # centers of own classes
        matched = np.concatenate([match_b, match_c])
        rest_mask = np.ones(B + C, bool)
        rest_mask[own] = False
        rest_mask[matched] = False
        rest = np.flatnonzero(rest_mask)
        perm = np.concatenate(
            [own, matched, rest,
             np.arange(B + C, N),                          # ood
             np.arange(N, NPAD)]                           # pad
        )
        assert perm.shape == (NPAD,)
        perms.append(perm)
        eqw_need = max(eqw_need, -(-(RPC + len(matched)) // 512))
        mm_need = max(mm_need, RPC + len(matched))

    eqw = max(eqw_need, 2)  # chunks that must carry matches (expected 2)
    wneed = min(-(-mm_need // 128) * 128, eqw * 512)  # matched extent, 128-aligned

    def tile_T(x):
        # [ncols, 512] -> [128, (ncols/512)*2048] in the SBUF chunk layout:
        # block ch at ch*2048, inner offset 512*ks + j  (ks = dim-slice, j = col)
        nch = x.shape[0] // 512
        xt = np.ascontiguousarray(x.T)                     # [512, ncols]
        return np.ascontiguousarray(
            xt.reshape(4, 128, nch, 512).transpose(1, 2, 0, 3).reshape(128, nch * 2048)
        )

    in_maps = []
    for c in range(NCORES):
        perm = perms[c]
        gT8_c = tile_T(g8[perm])
        fT8_c = tile_T(g8[perm[:RPC]])
        brow2 = np.stack([b_h[perm], b_l[perm]])           # [2, NPAD]
        cst_c = np.ascontiguousarray(
            np.concatenate([ones2, brow2], axis=1).astype(BFNP)
        )
        ta_p = lab[perm[:wneed]].astype(np.float32)
        ta_bc = np.ascontiguousarray(np.broadcast_to(ta_p, (128, wneed)))
        tvec_c = np.ascontiguousarray(
            t_sorted[RPC * c : RPC * (c + 1)].reshape(MT, 128).T.astype(np.float32)
        )
        in_maps.append(
            {
                "gT8": gT8_c,
                "fT8": fT8_c,
                "cst": cst_c,
                "ta": ta_bc,
                "tvec": tvec_c,
                "ident": ident,
            }
        )

    # effective per-class bias as the device psum sees it (fp32 add of the
    # bf16 hi/lo pair), descaled back to bias1 units
    cls_bias = -(np.log(w_full) + 10.0) / 10.0 * 256.0
    cb_h = cls_bias.astype(BFNP)
    cb_l = (cls_bias - cb_h.astype(np.float64)).astype(BFNP)
    bias_eff_cls = (
        cb_h.astype(np.float32).astype(np.float64)
        + cb_l.astype(np.float32).astype(np.float64)
    ) / 256.0

    host = {"t_sorted": t_sorted, "w_full": w_full, "bias_eff_cls": bias_eff_cls,
            "eqw": eqw, "wneed": wneed}
    return in_maps, host


def _combine(results, host):
    t_sorted = host["t_sorted"]
    w_full = host["w_full"]
    cnt_batch = np.bincount(t_sorted, minlength=C).astype(np.float64)

    A = np.empty(B)
    S2 = np.empty(B)
    S3 = np.empty(B)
    diag = np.empty(B)
    for c in range(NCORES):
        o = np.asarray(results[c]["out"], np.float64)  # [128, 16]
        for m in range(MT):
            rs = slice(RPC * c + 128 * m, RPC * c + 128 * (m + 1))
            A[rs] = o[:, 4 * m]
            S2[rs] = o[:, 4 * m + 1]
            S3[rs] = o[:, 4 * m + 2]
            diag[rs] = o[:, 4 * m + 3]

    ws = w_full[t_sorted]
    K = cnt_batch[t_sorted]
    ds_ = 1.0 / (ws - 1.0) - 1.0 / ws
    b1s = host["bias_eff_cls"][t_sorted]
    e1s = np.exp(10.0 / 256.0 * diag)
    S = A - e1s + ds_ * ws * (S2 - e1s)
    P = 10.0 * (S3 / 256.0 - K * b1s - diag / 256.0) - 10.0 * K
    val = P / K - np.log(S)
    return np.float32(-val.mean())


def _run(inputs, trace=False, **kw):
    in_maps, host = _prepare(**inputs)
    nc = _get_nc(host["eqw"], host["wneed"])
    res = run_bass_kernel_spmd(nc, in_maps, list(range(NCORES)), trace=trace, **kw)
    loss = _combine(res.results, host)
    return loss, res


def kernel(**inputs):
    loss, _ = _run(inputs)
    return loss


# revision 17
# speedup vs baseline: 1.0298x; 1.0298x over previous
"""Trainium2 Bass kernel for the class-balanced supervised-contrastive loss.

Math (reference semantics, shift-invariant form with constant shift 10):
  l_ij = (f_i . g_j) / T,  T = 0.1, g = [features; centers; features_ood]
  E_ij = exp(l_ij - 10)
  S_i  = sum_{j != i} E_ij / (w_j - eq_ij)        (w_j = class count, eq = label match)
  P_i  = sum_{j != i} eq_ij (l_ij - 10)
  loss = -mean_i( P_i / K_i - log S_i ),  K_i = batch count of class t_i

Device per row (tolerance is 2e-2 rel, so a single fp8 pass suffices —
validated 5e-5 end-to-end on the reference data):
  psum_ij = 256*(r_ij + bias1_j),  bias1_j = -(ln w_j + 10)/10
    r from one fp8(e4m3, inputs pre-scaled x16) DoubleRow matmul pair
    (K=256 per instruction, 0.5 PE cycles/row), bias via a K=2 bf16
    hi/lo ones-matmul into the same PSUM accumulation group.
  ACT: E1 = exp((10/256)*psum), accum_out -> A_i = sum_j E1  (all 18 chunks)
  DVE (window chunks only, where all label matches live after the host
  permutation): S2 = sum eq*E1, S3 = sum eq*psum, diag = psum_ii.
Everything else is O(B) host math.

Device layout per core (rows sharded, 512 rows/core): columns are permuted
to [own 512 rows | matched cols | rest | ood | pad] so every eq-match (and
the diagonal, at column 128m+p of chunk 0 for row-tile m partition p) is
confined to the first `eqw` (=2) 512-column chunks. Chunks are processed in
groups of 4 (one 4-bank PSUM tile, double-buffered) so each exp covers
[128, 2048] in one ACT instruction.
"""

import ml_dtypes
import numpy as np

import concourse.bass as bass
import concourse.mybir as mybir
import concourse.tile as tile
from concourse.bass_utils import run_bass_kernel_spmd

NCORES = 8
C, TEMP = 1000, 0.1
B, BO, D = 4096, 4096, 512
N = B + C + BO              # 9192
NPAD = 9216                 # 18 * 512
PAD = NPAD - N
NCH = NPAD // 512           # 18 column chunks
RPC = B // NCORES           # 512 rows per core
MT = RPC // 128             # 4 row tiles per core
SCALE = 16.0                # fp8 operand pre-scale; psum carries 256*(r+bias)
# chunk groups per ACT instruction; the window group (chunks 0..1, which
# feeds the extra DVE reductions) goes LAST so its psum->DVE->ACT chain
# hides under the other quads' ACT work at each m boundary
GRPS = [(2, 6), (6, 10), (10, 14), (14, 18), (0, 2)]

F32 = mybir.dt.float32
BF16 = mybir.dt.bfloat16
F8 = mybir.dt.float8e4
ALU = mybir.AluOpType
AF = mybir.ActivationFunctionType
BFNP = ml_dtypes.bfloat16
F8NP = ml_dtypes.float8_e4m3
DR = mybir.MatmulPerfMode.DoubleRow

# This walrus build accepts only one sync-wait command per engine instruction.
# Move surplus waits onto standalone EventSemaphore instructions just before
# the affected instruction (same engine, so blocking semantics are identical).
_SPLIT_SKIP = ("InstEventSemaphore",)


def _split_multi_waits(nc):
    n = 0
    for f in nc.m.functions:
        for bb in f.blocks:
            new = []
            for ins in bb.instructions:
                si = ins.sync_info
                if (
                    si is not None
                    and si.on_wait
                    and len(si.on_wait) > 1
                    and type(ins).__name__ not in _SPLIT_SKIP
                ):
                    waits = list(si.on_wait)
                    for w in waits[:-1]:
                        es = mybir.InstEventSemaphore(
                            name=f"wsplit_{n}",
                            engine=ins.engine,
                            sync_info=mybir.SyncInfo(on_wait=[w], on_update=[]),
                        )
                        n += 1
                        new.append(es)
                    ins.sync_info = mybir.SyncInfo(
                        on_wait=[waits[-1]], on_update=list(si.on_update)
                    )
                new.append(ins)
            bb.instructions = new
    return n


def _build_nc(eqw=2, wneed=1024):
    assert eqw <= 2, "label window must fit the first (0,2) chunk group"
    wcols = wneed  # actual matched extent (<= eqw*512), 128-aligned
    nc = bass.Bass()
    # host pre-tiles to the SBUF layout: chunk ch at [128, 4, 512] block ch,
    # element (p, ks, j) = scaled g[col j of chunk][dim p + 128*ks]
    gT8 = nc.declare_dram_parameter("gT8", [128, NCH * 2048], F8, isOutput=False)
    fT8 = nc.declare_dram_parameter("fT8", [128, 2048], F8, isOutput=False)
    # [2, .]: row 0 = (ones128, bias_hi row), row 1 = lo parts
    cst = nc.declare_dram_parameter("cst", [2, 128 + NPAD], BF16, isOutput=False)
    ta = nc.declare_dram_parameter("ta", [128, wcols], F32, isOutput=False)
    tvec = nc.declare_dram_parameter("tvec", [128, MT], F32, isOutput=False)
    ident = nc.declare_dram_parameter("ident", [128, 128], F32, isOutput=False)
    out = nc.declare_dram_parameter("out", [128, 4 * MT], F32, isOutput=True)

    with tile.TileContext(nc) as tc:
        with (
            tc.tile_pool(name="const", bufs=1) as const,
            tc.tile_pool(name="stats", bufs=1) as stats,
            tc.tile_pool(name="e1", bufs=2) as e1p,
            tc.tile_pool(name="scr", bufs=2) as scr,
            tc.tile_pool(name="psum", bufs=2, space="PSUM") as psp,
        ):
            # DMAs split across the SP and Pool (SWDGE) queues: the tile-sim
            # charges transfer time (free-dim bytes) serially per issuing
            # engine, so one queue would cost ~24us of latency.
            ft8 = const.tile([128, 4, 512], F8)
            nc.sync.dma_start(out=ft8[:], in_=fT8[:])
            cst_sb = const.tile([2, 128 + NPAD], BF16)
            g8 = const.tile([128, NCH * 4, 512], F8)

            def g8dma(eng, c0, c1):
                eng.dma_start(
                    out=g8[:, 4 * c0 : 4 * c1, :], in_=gT8[:, 2048 * c0 : 2048 * c1]
                )

            def cstdma(eng, a, b):
                nc_off = 128  # ones row precedes the biases
                eng.dma_start(
                    out=cst_sb[:, nc_off + 512 * a : nc_off + 512 * b],
                    in_=cst[:, nc_off + 512 * a : nc_off + 512 * b],
                )

            # DMAs split across the SP and Pool (SWDGE) queues — the tile-sim
            # charges transfer time (free-dim bytes) serially per issuing
            # engine — and sequenced so each piece lands just before the
            # group order [2-5, 6-9, 10-13, 14-17, 0-1] consumes it.
            nc.gpsimd.dma_start(out=cst_sb[:, 0:128], in_=cst[:, 0:128])  # ones
            cstdma(nc.gpsimd, 2, 6)
            g8dma(nc.sync, 2, 4)
            g8dma(nc.sync, 4, 6)
            g8dma(nc.gpsimd, 6, 8)
            g8dma(nc.gpsimd, 8, 10)
            g8dma(nc.sync, 10, 12)
            cstdma(nc.gpsimd, 6, 10)
            g8dma(nc.sync, 12, 14)
            cstdma(nc.gpsimd, 10, 14)
            tvec_sb = const.tile([128, MT], F32)
            nc.sync.dma_start(out=tvec_sb[:], in_=tvec[:])
            ident_sb = const.tile([128, 128], F32)
            nc.sync.dma_start(out=ident_sb[:], in_=ident[:])
            g8dma(nc.gpsimd, 14, 16)
            g8dma(nc.gpsimd, 16, 18)
            g8dma(nc.sync, 0, 2)
            cstdma(nc.sync, 14, 18)
            cstdma(nc.sync, 0, 2)
            ta_sb = const.tile([128, wcols], F32)
            nc.gpsimd.dma_start(out=ta_sb[:], in_=ta[:])
            ones_sb = cst_sb[:, 0:128]
            brow_sb = cst_sb[:, 128 : 128 + NPAD]

            outsb = stats.tile([128, 4 * MT], F32)
            a_slot = [stats.tile([128, len(GRPS)], F32, name=f"a{m}") for m in range(MT)]

            # warm the ACT Exp table during the DMA head so the first real
            # activation doesn't pay the 1.3us table load
            warm = stats.tile([2, 1], F32)
            nc.scalar.activation(warm[:], ones_sb[:, 0:1], AF.Exp, scale=1.0)

            for m in range(MT):
                for gi, (gs, ge) in enumerate(GRPS):
                    gw = (ge - gs) * 512
                    ps = psp.tile([128, 2048], F32)
                    for ch in range(gs, ge):
                        co = (ch - gs) * 512
                        pslice = ps[:, co : co + 512]
                        # bias matmul last so the fp8 work can start before
                        # the bias rows finish streaming in
                        for q in range(2):
                            nc.tensor.matmul(
                                pslice,
                                ft8[:, 2 * q : 2 * q + 2, 128 * m : 128 * (m + 1)],
                                g8[:, 4 * ch + 2 * q : 4 * ch + 2 * q + 2, :],
                                start=(q == 0),
                                stop=False,
                                perf_mode=DR,
                            )
                        nc.tensor.matmul(
                            pslice,
                            ones_sb,
                            brow_sb[:, 512 * ch : 512 * (ch + 1)],
                            start=False,
                            stop=True,
                        )
                    if gs == 0:
                        # psum readers (S3, diag) are emitted BEFORE the exp:
                        # tile accesses chain in emission order, so putting
                        # them first lets them start as soon as the PE's
                        # window psum is ready (the exp then chains after).
                        sc3 = scr.tile([128, wcols], F32, tag="scr3")
                        nc.vector.scalar_tensor_tensor(
                            out=sc3[:],
                            in0=ta_sb[:],
                            scalar=tvec_sb[:, m : m + 1],
                            in1=ps[:, :wcols],
                            op0=ALU.is_equal,
                            op1=ALU.mult,
                            accum_out=outsb[:, 4 * m + 2 : 4 * m + 3],
                        )
                        # local row p's own column is chunk-0 column 128m+p,
                        # so the psum diagonal of this [128,128] sub-block is
                        # the self dot-product (plus bias) bit-exactly.
                        sd = scr.tile([128, 128], F32, tag="scrd")
                        nc.vector.scalar_tensor_tensor(
                            out=sd[:],
                            in0=ident_sb[:],
                            scalar=1.0,
                            in1=ps[:, 128 * m : 128 * (m + 1)],
                            op0=ALU.mult,
                            op1=ALU.mult,
                            accum_out=outsb[:, 4 * m + 3 : 4 * m + 4],
                        )
                    # exp in place over the psum tile for plain groups
                    # (saves the SBUF write and the e1 buffer); the window
                    # group writes e1 to SBUF so its psum frees right after
                    # the ACT instead of waiting for the S2 reduction
                    if gs == 0:
                        e1 = e1p.tile([128, 1024], F32, name="e1", tag="e1")
                        eout = e1[:, :gw]
                    else:
                        eout = ps[:, :gw]
                    nc.scalar.activation(
                        eout,
                        ps[:, :gw],
                        AF.Exp,
                        scale=10.0 / 256.0,
                        accum_out=a_slot[m][:, gi : gi + 1],
                    )
                    if gs == 0:
                        sc = scr.tile([128, wcols], F32, tag="scr2")
                        nc.vector.scalar_tensor_tensor(
                            out=sc[:],
                            in0=ta_sb[:],
                            scalar=tvec_sb[:, m : m + 1],
                            in1=e1[:, :wcols],
                            op0=ALU.is_equal,
                            op1=ALU.mult,
                            accum_out=outsb[:, 4 * m + 1 : 4 * m + 2],
                        )
                nc.vector.tensor_reduce(
                    outsb[:, 4 * m : 4 * m + 1],
                    a_slot[m][:],
                    mybir.AxisListType.X,
                    ALU.add,
                )
            nc.sync.dma_start(out=out[:], in_=outsb[:])
    _split_multi_waits(nc)
    return nc


_nc_by_cfg = {}


def _get_nc(eqw, wneed):
    key = (eqw, wneed)
    if key not in _nc_by_cfg:
        _nc_by_cfg[key] = _build_nc(eqw, wneed)
    return _nc_by_cfg[key]


def _prepare(centers1, features, targets, features_ood, pseudo_target_ood):
    """Host-side O(N log N) prep.

    Rows are globally sorted by class and sharded contiguously, so each
    core's 512 rows cover ~C/8 classes whose other members mostly live in
    the same core. Per core the g columns are permuted to
    [own 512 rows | all other same-class batch cols + own-class centers |
     rest bc cols | ood | pad], which confines every eq-match (and the
    diagonal, at column 128m+p for row-tile m partition p) to the first
    eqw chunks. Only those chunks need the masked S2/S3 reductions.
    """
    centers1 = np.asarray(centers1, np.float32)
    features = np.asarray(features, np.float32)
    features_ood = np.asarray(features_ood, np.float32)
    targets = np.asarray(targets).astype(np.int64)
    pseudo = np.asarray(pseudo_target_ood).astype(np.int64)

    tac = np.concatenate([targets, np.arange(C), pseudo])
    w_full = np.bincount(tac, minlength=C).astype(np.float64)

    # class-id label per g row (incl. centers/ood), and bias per g row
    lab = np.concatenate([targets, np.arange(C), np.full(BO, C, np.int64),
                          np.full(PAD, -1, np.int64)])
    bias1 = np.full(NPAD, -20.0 * 256.0, np.float64)
    bias1[:N] = -(np.log(w_full[tac]) + 10.0) / 10.0 * 256.0
    b_h = bias1.astype(BFNP)
    b_l = (bias1 - b_h.astype(np.float64)).astype(BFNP)

    g = np.concatenate(
        [features, centers1, features_ood, np.zeros((PAD, D), np.float32)], axis=0
    )
    g8 = (g * SCALE).astype(F8NP)

    row_perm = np.argsort(targets, kind="stable")
    t_sorted = targets[row_perm]

    ident = np.eye(128, dtype=np.float32)
    ones2 = np.ones((2, 128), BFNP)

    # per-core column permutations
    perms = []
    eqw_need = 1
    mm_need = RPC + 1
    all_batch = np.arange(B)
    for c in range(NCORES):
        own = row_perm[RPC * c : RPC * (c + 1)]            # sorted by class
        tset = np.zeros(C + 1, bool)
        tset[t_sorted[RPC * c : RPC * (c + 1)]] = True
        in_own = np.zeros(B, bool)
        in_own[own] = True
        match_b = all_batch[tset[targets] & ~in_own]       # other cores' rows, own classes
        match_c = B + np.flatnonzero(tset[:C])             # centers of own classes
        matched = np.concatenate([match_b, match_c])
        rest_mask = np.ones(B + C, bool)
        rest_mask[own] = False
        rest_mask[matched] = False
        rest = np.flatnonzero(rest_mask)
        perm = np.concatenate(
            [own, matched, rest,
             np.arange(B + C, N),                          # ood
             np.arange(N, NPAD)]                           # pad
        )
        assert perm.shape == (NPAD,)
        perms.append(perm)
        eqw_need = max(eqw_need, -(-(RPC + len(matched)) // 512))
        mm_need = max(mm_need, RPC + len(matched))

    eqw = max(eqw_need, 2)  # chunks that must carry matches (expected 2)
    wneed = min(-(-mm_need // 128) * 128, eqw * 512)  # matched extent, 128-aligned

    def tile_T(x):
        # [ncols, 512] -> [128, (ncols/512)*2048] in the SBUF chunk layout:
        # block ch at ch*2048, inner offset 512*ks + j  (ks = dim-slice, j = col)
        nch = x.shape[0] // 512
        xt = np.ascontiguousarray(x.T)                     # [512, ncols]
        return np.ascontiguousarray(
            xt.reshape(4, 128, nch, 512).transpose(1, 2, 0, 3).reshape(128, nch * 2048)
        )

    in_maps = []
    for c in range(NCORES):
        perm = perms[c]
        gT8_c = tile_T(g8[perm])
        fT8_c = tile_T(g8[perm[:RPC]])
        brow2 = np.stack([b_h[perm], b_l[perm]])           # [2, NPAD]
        cst_c = np.ascontiguousarray(
            np.concatenate([ones2, brow2], axis=1).astype(BFNP)
        )
        ta_p = lab[perm[:wneed]].astype(np.float32)
        ta_bc = np.ascontiguousarray(np.broadcast_to(ta_p, (128, wneed)))
        tvec_c = np.ascontiguousarray(
            t_sorted[RPC * c : RPC * (c + 1)].reshape(MT, 128).T.astype(np.float32)
        )
        in_maps.append(
            {
                "gT8": gT8_c,
                "fT8": fT8_c,
                "cst": cst_c,
                "ta": ta_bc,
                "tvec": tvec_c,
                "ident": ident,
            }
        )

    # effective per-class bias as the device psum sees it (fp32 add of the
    # bf16 hi/lo pair), descaled back to bias1 units
    cls_bias = -(np.log(w_full) + 10.0) / 10.0 * 256.0
    cb_h = cls_bias.astype(BFNP)
    cb_l = (cls_bias - cb_h.astype(np.float64)).astype(BFNP)
    bias_eff_cls = (
        cb_h.astype(np.float32).astype(np.float64)
        + cb_l.astype(np.float32).astype(np.float64)
    ) / 256.0

    host = {"t_sorted": t_sorted, "w_full": w_full, "bias_eff_cls": bias_eff_cls,
            "eqw": eqw, "wneed": wneed}
    return in_maps, host


def _combine(results, host):
    t_sorted = host["t_sorted"]
    w_full = host["w_full"]
    cnt_batch = np.bincount(t_sorted, minlength=C).astype(np.float64)

    A = np.empty(B)
    S2 = np.empty(B)
    S3 = np.empty(B)
    diag = np.empty(B)
    for c in range(NCORES):
        o = np.asarray(results[c]["out"], np.float64)  # [128, 16]
        for m in range(MT):
            rs = slice(RPC * c + 128 * m, RPC * c + 128 * (m + 1))
            A[rs] = o[:, 4 * m]
            S2[rs] = o[:, 4 * m + 1]
            S3[rs] = o[:, 4 * m + 2]
            diag[rs] = o[:, 4 * m + 3]

    ws = w_full[t_sorted]
    K = cnt_batch[t_sorted]
    ds_ = 1.0 / (ws - 1.0) - 1.0 / ws
    b1s = host["bias_eff_cls"][t_sorted]
    e1s = np.exp(10.0 / 256.0 * diag)
    S = A - e1s + ds_ * ws * (S2 - e1s)
    P = 10.0 * (S3 / 256.0 - K * b1s - diag / 256.0) - 10.0 * K
    val = P / K - np.log(S)
    return np.float32(-val.mean())


def _run(inputs, trace=False, **kw):
    in_maps, host = _prepare(**inputs)
    nc = _get_nc(host["eqw"], host["wneed"])
    res = run_bass_kernel_spmd(nc, in_maps, list(range(NCORES)), trace=trace, **kw)
    loss = _combine(res.results, host)
    return loss, res


def kernel(**inputs):
    loss, _ = _run(inputs)
    return loss
